# revision 8
# baseline (speedup 1.0000x reference)
"""Bass/Tile TRN2 kernel for nn_Link_83047487635827 (gnn_message_passing).

Math (verified against the reference):
    binary = (tag_to_token > 0)                       # (T, N)
    temp   = relu(C^T @ binary),  C = I - strict_lower_ones(T)
    r      = rowsum(temp); P = temp @ inputs          # (T,), (T, D)
    child  == gat_mask  (reference deduce_child is an identity for 0/1 masks)
    out    = (I - S_up)^{-1} @ L_low @ diag(1/r) @ P
    (I - S_up)^{-1} = prod_{k=0..6} (I + S_up^(2^k))   # S_up nilpotent

Sharding (tensor parallel over D, per the hint): every core loads the FULL
binarized tag_to_token and redundantly computes temp, but only its own
128-column slice of x (bf16) and P.  A ones-column appended to x yields r in
the same PSUM accumulation.  No collective at all; each core writes its
(T, 128) output slice and the host concatenates.

binary and C are 0/+-1, so the temp matmul is EXACT in fp8e4 — and with the
DoubleRow perf mode (both operands laid out (64, 2, .), contraction over
2x64) it runs at 0.5 PE cycles/row.  temp is 0/1, exact in bf16 for the P
matmul; the only approximation is bf16 rounding of x (~0.15% << the 2e-2
tolerance).  The relu+PSUM-drain of temp alternates between the DVE and
Activation engines so neither becomes the bottleneck.
"""

import numpy as np

B, T, N, D = 1, 128, 32768, 1024
NCORES = 8
DS = D // NCORES          # output columns per core = 128
CHUNK = 512               # tokens per pipeline chunk
NCHUNK = N // CHUNK       # 64
SUBS = CHUNK // 128       # 4 token-subtiles of 128 per chunk
XW = DS + 1               # x slice + ones column = 129
NSUB = N // 128           # 256 subtiles total

_PROGRAM = {}             # (with_cc, loop_stream) -> nc


def _host_consts():
    import ml_dtypes

    f32 = np.float32
    ident = np.eye(T, dtype=f32)
    # C[j, i] = 1 if j == i, -1 if j > i  (temp^T tile = binary_tile^T @ C)
    cmat = np.eye(T, dtype=f32) - np.tril(np.ones((T, T), dtype=f32), -1)
    # DoubleRow layout: [p, i, n] = cmat[i*64 + p, n]
    cmat_dr = np.ascontiguousarray(
        cmat.reshape(2, 64, T).transpose(1, 0, 2)
    ).astype(ml_dtypes.float8_e4m3)
    msl = np.tril(np.ones((T, T), dtype=f32), -1)   # strict lower
    msu = np.triu(np.ones((T, T), dtype=f32), 1)    # strict upper
    mle = np.tril(np.ones((T, T), dtype=f32), 0)    # lower inclusive
    return {
        "ident": ident, "cmat": cmat_dr,
        "msl": msl, "msu": msu, "mle": mle,
    }


def _build_program(with_cc=True, loop_stream=1):
    import contextlib

    import concourse.bacc as bacc
    import concourse.bass as bass
    import concourse.mybir as mybir
    import concourse.tile as tile
    from concourse.bass import ts

    f32 = mybir.dt.float32
    bf16 = mybir.dt.bfloat16
    fp8 = mybir.dt.float8e4
    i32 = mybir.dt.int32
    Alu = mybir.AluOpType
    DR = mybir.MatmulPerfMode.DoubleRow
    Relu = mybir.ActivationFunctionType.Relu

    nc = bacc.Bacc(
        "TRN2", target_bir_lowering=False, debug=False, num_devices=NCORES
    )

    # x slice, host-permuted: row j*128+p, col s*XW+q = x_aug[(j*SUBS+s)*128+p, q]
    xs_d = nc.dram_tensor("xs", (NCHUNK * 128, SUBS * XW), bf16, kind="ExternalInput")
    # DoubleRow layout: [p, i, tok] = binary[i*64 + p, tok]
    t2t_d = nc.dram_tensor("t2t", (64, 2, N), fp8, kind="ExternalInput")
    gm_d = nc.dram_tensor("gm", (T, T), i32, kind="ExternalInput")
    ident_d = nc.dram_tensor("ident", (T, T), f32, kind="ExternalInput")
    cmat_d = nc.dram_tensor("cmat", (64, 2, T), fp8, kind="ExternalInput")
    msl_d = nc.dram_tensor("msl", (T, T), f32, kind="ExternalInput")
    msu_d = nc.dram_tensor("msu", (T, T), f32, kind="ExternalInput")
    mle_d = nc.dram_tensor("mle", (T, T), f32, kind="ExternalInput")
    out_d = nc.dram_tensor("out", (T, DS), f32, kind="ExternalOutput")

    with tile.TileContext(nc) as tc:
        with (
            tc.tile_pool(name="const", bufs=1) as constp,
            tc.tile_pool(name="xin", bufs=4) as xp,
            tc.tile_pool(name="t2tin", bufs=4) as t2tp,
            tc.tile_pool(name="work", bufs=4) as workp,
            tc.tile_pool(name="mchain", bufs=2) as mp,
            tc.tile_pool(name="psacc", bufs=1, space=bass.MemorySpace.PSUM) as psA,
            tc.tile_pool(name="pstt", bufs=2, space=bass.MemorySpace.PSUM) as psB,
            tc.tile_pool(name="psm", bufs=3, space=bass.MemorySpace.PSUM) as psM,
        ):
            # ---- constants ----
            ident = constp.tile([T, T], f32, tag="ident")
            nc.sync.dma_start(ident[:], ident_d[:])
            cmat = constp.tile([64, 2, T], fp8, tag="cmat")
            nc.sync.dma_start(cmat[:], cmat_d[:])
            msl = constp.tile([T, T], f32, tag="msl")
            nc.sync.dma_start(msl[:], msl_d[:])
            msu = constp.tile([T, T], f32, tag="msu")
            nc.sync.dma_start(msu[:], msu_d[:])
            mle = constp.tile([T, T], f32, tag="mle")
            nc.sync.dma_start(mle[:], mle_d[:])
            gm_i = constp.tile([T, T], i32, tag="gmi")
            nc.sync.dma_start(gm_i[:], gm_d[:])
            gm_f = constp.tile([T, T], f32, tag="gmf")
            nc.vector.tensor_copy(gm_f[:], gm_i[:])

            # ---- recurrence matrix chain (tiny; overlaps the stream loop) ----
            gmT_ps = psM.tile([T, T], f32, tag="mm")
            nc.tensor.transpose(gmT_ps[:], gm_f[:], ident[:])
            gmT = constp.tile([T, T], f32, tag="gmT")
            nc.vector.tensor_copy(gmT[:], gmT_ps[:])

            Tp = mp.tile([T, T], f32, tag="Tp")
            nc.vector.tensor_tensor(out=Tp[:], in0=gmT[:], in1=msl[:], op=Alu.mult)
            TpT = mp.tile([T, T], f32, tag="TpT")
            nc.vector.tensor_tensor(out=TpT[:], in0=gm_f[:], in1=msu[:], op=Alu.mult)
            G = mp.tile([T, T], f32, tag="G")
            nc.vector.tensor_tensor(out=G[:], in0=ident[:], in1=Tp[:], op=Alu.add)
            L_low = constp.tile([T, T], f32, tag="Llow")
            nc.vector.tensor_tensor(out=L_low[:], in0=gm_f[:], in1=mle[:], op=Alu.mult)

            for _k in range(1, 7):
                # matmul(out, lhsT, rhs) = lhsT.T @ rhs
                sq_ps = psM.tile([T, T], f32, tag="mm")
                nc.tensor.matmul(sq_ps[:], Tp[:], TpT[:])      # (Tp^2)^T
                sq2_ps = psM.tile([T, T], f32, tag="mm")
                nc.tensor.matmul(sq2_ps[:], TpT[:], Tp[:])     # Tp^2
                Tp_n = mp.tile([T, T], f32, tag="Tp")
                nc.vector.tensor_copy(Tp_n[:], sq2_ps[:])
                TpT_n = mp.tile([T, T], f32, tag="TpT")
                nc.vector.tensor_copy(TpT_n[:], sq_ps[:])
                gu_ps = psM.tile([T, T], f32, tag="mm")
                nc.tensor.matmul(gu_ps[:], TpT_n[:], G[:])     # Tp^2 @ G
                G_n = mp.tile([T, T], f32, tag="G")
                nc.vector.tensor_tensor(out=G_n[:], in0=G[:], in1=gu_ps[:], op=Alu.add)
                Tp, TpT, G = Tp_n, TpT_n, G_n

            mt_ps = psM.tile([T, T], f32, tag="mm")
            nc.tensor.matmul(mt_ps[:], L_low[:], G[:])         # M^T = L_low^T @ G
            MT = constp.tile([T, T], f32, tag="MT")
            nc.vector.tensor_copy(MT[:], mt_ps[:])

            # ---- streaming phase: P_aug[tag, :DS] += temp @ x_slice,
            #      P_aug[tag, DS] += rowsum(temp) via the ones column ----
            loop_cm = (
                tc.For_i(0, loop_stream, 1)
                if loop_stream > 1
                else contextlib.nullcontext()
            )
            with loop_cm:
                P_ps = psA.tile([128, XW], f32, tag="pacc")

                for j in range(NCHUNK):
                    tt_in = t2tp.tile([64, 2, CHUNK], fp8, tag="ttin")
                    nc.sync.dma_start(tt_in[:], t2t_d[:, :, ts(j, CHUNK)])
                    xt = xp.tile([128, SUBS * XW], bf16, tag="xt")
                    nc.sync.dma_start(xt[:], xs_d[ts(j, 128), :])

                    ttp = psB.tile([128, CHUNK], f32, tag="tt")
                    for s in range(SUBS):
                        nc.tensor.matmul(
                            ttp[:, ts(s, 128)],
                            tt_in[:, :, ts(s, 128)],
                            cmat[:],
                            perf_mode=DR,
                        )
                    tempT = workp.tile([128, CHUNK], bf16, tag="tempT")
                    if j % 2 == 0:
                        nc.vector.tensor_scalar_max(tempT[:], ttp[:], 0.0)
                    else:
                        nc.scalar.activation(tempT[:], ttp[:], Relu)

                    for s in range(SUBS):
                        i = j * SUBS + s
                        nc.tensor.matmul(
                            P_ps[:],
                            tempT[:, ts(s, 128)],
                            xt[:, ts(s, XW)],
                            start=(i == 0),
                            stop=(i == NSUB - 1),
                        )

            # ---- out = M @ (diag(1/r) P)  (lhsT = MT) ----
            P_sb = workp.tile([128, XW], f32, tag="Psb")
            nc.vector.tensor_copy(P_sb[:], P_ps[:])
            inv_r = workp.tile([128, 1], f32, tag="invr")
            nc.vector.reciprocal(inv_r[:], P_sb[:, DS : DS + 1])
            nc.vector.tensor_scalar_mul(P_sb[:, 0:DS], P_sb[:, 0:DS], inv_r[:])

            o_ps = psB.tile([128, CHUNK], f32, tag="tt")
            nc.tensor.matmul(o_ps[:, 0:DS], MT[:], P_sb[:, 0:DS])
            out_sb = workp.tile([128, DS], f32, tag="outsb")
            nc.vector.tensor_copy(out_sb[:], o_ps[:, 0:DS])
            nc.sync.dma_start(out_d[:], out_sb[:])

    nc.compile()
    return nc


def _get_program(with_cc=True, loop_stream=1):
    key = (with_cc, loop_stream)
    if key not in _PROGRAM:
        _PROGRAM[key] = _build_program(with_cc, loop_stream)
    return _PROGRAM[key]


def _make_in_maps(inputs):
    import ml_dtypes

    bf16 = ml_dtypes.bfloat16
    x = np.asarray(inputs["inputs"], dtype=np.float32).reshape(N, D)
    t2t = np.asarray(inputs["tag_to_token"], dtype=np.float32).reshape(T, N)
    gm = np.asarray(inputs["gat_mask"], dtype=np.int32).reshape(T, T)
    # DoubleRow layout: [p, i, tok] = binary[i*64 + p, tok]
    t2t_bin = np.ascontiguousarray(
        (t2t > 0).reshape(2, 64, N).transpose(1, 0, 2)
    ).astype(ml_dtypes.float8_e4m3)
    consts = _host_consts()
    in_maps = []
    for c in range(NCORES):
        xc = x[:, c * DS : (c + 1) * DS].astype(bf16)
        xa = np.concatenate([xc, np.ones((N, 1), dtype=bf16)], axis=1)
        # (N, XW) -> chunk-major layout: [j*128+p, s*XW+q] = xa[(j*SUBS+s)*128+p, q]
        xa = np.ascontiguousarray(
            xa.reshape(NCHUNK, SUBS, 128, XW)
            .transpose(0, 2, 1, 3)
            .reshape(NCHUNK * 128, SUBS * XW)
        )
        m = {"xs": xa, "t2t": t2t_bin, "gm": gm}
        m.update(consts)
        in_maps.append(m)
    return in_maps


def _run(inputs, trace=False, **kw):
    from concourse.bass_utils import run_bass_kernel_spmd

    nc = _get_program()
    in_maps = _make_in_maps(inputs)
    res = run_bass_kernel_spmd(
        nc, in_maps, list(range(NCORES)), trace=trace, **kw
    )
    out = np.empty((T, D), dtype=np.float32)
    for c in range(NCORES):
        out[:, c * DS : (c + 1) * DS] = np.asarray(res.results[c]["out"])
    return out.reshape(B, T, D), res


def kernel(**inputs) -> np.ndarray:
    out, _ = _run(inputs, trace=False)
    return out


# revision 52
# speedup vs baseline: 1.0000x; 1.0000x over previous
"""Bass/Tile TRN2 kernel for nn_Link_83047487635827 (gnn_message_passing).

Math (verified against the reference):
    binary = (tag_to_token > 0)                       # (T, N)
    temp   = relu(C^T @ binary),  C = I - strict_lower_ones(T)
    r      = rowsum(temp); P = temp @ inputs          # (T,), (T, D)
    child  == gat_mask  (reference deduce_child is an identity for 0/1 masks)
    out    = (I - S_up)^{-1} @ L_low @ diag(1/r) @ P
    (I - S_up)^{-1} = prod_{k=0..6} (I + S_up^(2^k))   # S_up nilpotent

Sharding (tensor parallel over D, per the hint): every core loads the FULL
binarized tag_to_token and redundantly computes temp, but only its own
128-column slice of x (bf16) and P.  A ones-column appended to x yields r in
the same PSUM accumulation.  No collective at all; each core writes its
(T, 128) output slice and the host concatenates.

binary and C are 0/+-1, so the temp matmul is EXACT in fp8e4, and with the
DoubleRow perf mode (operands laid out (64, 2, .), contraction 2x64) it runs
at 0.5 PE cycles/row.  temp is 0/1, exact in bf16 for the P matmul; the only
approximation is bf16 rounding of x (~0.15% << the 2e-2 tolerance).

Structure (phase-split to decouple the relu drain from the P matmuls):
  Phase A: stream t2t, temp matmuls, relu+PSUM-drain split across the
           DVE/Act/Pool engines into a RESIDENT (128, N) bf16 tempT buffer.
           The recurrence-matrix chain drips one op per chunk through the
           idle PE/Pool slack.
  Phase B: 256 back-to-back P matmuls accumulating (T, 129) PSUM, gated
           only by the prefetched xs span DMAs.
"""

import numpy as np

B, T, N, D = 1, 128, 32768, 1024
NCORES = 8
DS = D // NCORES          # output columns per core = 128
XW = DS + 1               # x slice + ones column = 129
NSUB = N // 128           # 256 subtiles total
CHUNK = 512               # phase-A tokens per chunk
NCHUNK = N // CHUNK       # 64
SUBS = CHUNK // 128       # 4
DSPAN = 8                 # chunks per t2t DMA span
NSPAN = NCHUNK // DSPAN   # 8 t2t span DMAs
DSPAN_X = 4               # chunks per xs DMA span
NSPAN_X = NCHUNK // DSPAN_X  # 16 xs span DMAs
PSB = 5                   # ttp PSUM bufs
NENG = 2                  # relu engines (DVE/Act: GPSIMD cannot read PSUM)
# chain drip schedule inside phase B: step k ops at P-matmul indices
# base+0 (mms), +8 (copies), +20 (mmg), +28 (addg), stride 36
CHAIN_SCHED = {}
for _k in range(6):
    for _o, _off in enumerate((0, 8, 20, 28)):
        CHAIN_SCHED[_k * 36 + _off] = _k * 4 + _o
CHAIN_SCHED[224] = 24
CHAIN_SCHED[232] = 25

_PROGRAM = {}             # (with_cc, loop_stream) -> nc
_VARIANT = set()          # timing-analysis knobs, empty in production


def _host_consts():
    import ml_dtypes

    f32 = np.float32
    ident = np.eye(T, dtype=f32)
    # C[j, i] = 1 if j == i, -1 if j > i  (temp^T tile = binary_tile^T @ C)
    cmat = np.eye(T, dtype=f32) - np.tril(np.ones((T, T), dtype=f32), -1)
    # DoubleRow layout: [p, i, n] = cmat[i*64 + p, n]
    cmat_dr = np.ascontiguousarray(
        cmat.reshape(2, 64, T).transpose(1, 0, 2)
    ).astype(ml_dtypes.float8_e4m3)
    msl = np.tril(np.ones((T, T), dtype=f32), -1)   # strict lower
    msu = np.triu(np.ones((T, T), dtype=f32), 1)    # strict upper
    mle = np.tril(np.ones((T, T), dtype=f32), 0)    # lower inclusive
    # one DMA for all four (T, T) f32 masks: [ident | msl | msu | mle]
    cpack = np.ascontiguousarray(
        np.concatenate([ident, msl, msu, mle], axis=1)
    )
    return {"cpack": cpack, "cmat": cmat_dr}


def _build_program(with_cc=True, loop_stream=1):
    import contextlib

    import concourse.bacc as bacc
    import concourse.bass as bass
    import concourse.mybir as mybir
    import concourse.tile as tile
    from concourse.bass import ts

    f32 = mybir.dt.float32
    bf16 = mybir.dt.bfloat16
    fp8 = mybir.dt.float8e4
    i32 = mybir.dt.int32
    Alu = mybir.AluOpType
    DR = mybir.MatmulPerfMode.DoubleRow
    Relu = mybir.ActivationFunctionType.Relu
    Copy = mybir.ActivationFunctionType.Copy

    nc = bacc.Bacc(
        "TRN2", target_bir_lowering=False, debug=False, num_devices=NCORES
    )

    # x slice, host-permuted by DMA span: row g*128+p, col (c*SUBS+s)*XW+q =
    # x_aug[((g*DSPAN_X+c)*SUBS+s)*128+p, q]
    xs_d = nc.dram_tensor(
        "xs", (NSPAN_X * 128, DSPAN_X * SUBS * XW), bf16, kind="ExternalInput"
    )
    # DoubleRow layout: [p, i, tok] = binary[i*64 + p, tok]
    t2t_d = nc.dram_tensor("t2t", (64, 2, N), fp8, kind="ExternalInput")
    gm_d = nc.dram_tensor("gm", (T, T), i32, kind="ExternalInput")
    # packed f32 masks: [ident | msl | msu | mle]
    cpack_d = nc.dram_tensor("cpack", (T, 4 * T), f32, kind="ExternalInput")
    cmat_d = nc.dram_tensor("cmat", (64, 2, T), fp8, kind="ExternalInput")
    out_d = nc.dram_tensor("out", (T, DS), f32, kind="ExternalOutput")

    with tile.TileContext(nc) as tc:
        with (
            tc.tile_pool(name="const", bufs=1) as constp,
            tc.tile_pool(name="xin", bufs=NSPAN) as xp,
            tc.tile_pool(name="t2tin", bufs=NSPAN) as t2tp,
            tc.tile_pool(name="work", bufs=1) as workp,
            tc.tile_pool(name="mchain", bufs=2) as mp,
            tc.tile_pool(name="psacc", bufs=1, space=bass.MemorySpace.PSUM) as psA,
            tc.tile_pool(name="pstt", bufs=PSB, space=bass.MemorySpace.PSUM) as psB,
            tc.tile_pool(name="psm", bufs=2, space=bass.MemorySpace.PSUM) as psM,
        ):
            # ---- DMAs: cmat first (gates the first temp matmul), then the
            #      t2t spans (gate phase A), xs spans, small consts last ----
            cmat = constp.tile([64, 2, T], fp8, tag="cmat")
            nc.sync.dma_start(cmat[:], cmat_d[:])
            tt_tiles, xt_tiles = [], []
            for g in range(NSPAN):
                tt_in = t2tp.tile(
                    [64, 2, DSPAN * CHUNK], fp8, tag="ttin", name=f"tt{g}"
                )
                if "nodma" not in _VARIANT:
                    nc.sync.dma_start(tt_in[:], t2t_d[:, :, ts(g, DSPAN * CHUNK)])
                tt_tiles.append(tt_in)
            cpk = constp.tile([T, 4 * T], f32, tag="cpack")
            nc.sync.dma_start(cpk[:], cpack_d[:])
            ident, msl, msu, mle = (cpk[:, ts(k, T)] for k in range(4))
            gm_i = constp.tile([T, T], i32, tag="gmi")
            nc.sync.dma_start(gm_i[:], gm_d[:])
            gm_f = constp.tile([T, T], f32, tag="gmf")
            nc.vector.tensor_copy(gm_f[:], gm_i[:])
            for g in range(NSPAN_X):
                xt = xp.tile(
                    [128, DSPAN_X * SUBS * XW], bf16, tag="xt", name=f"xt{g}"
                )
                if "nodma" not in _VARIANT:
                    nc.sync.dma_start(xt[:], xs_d[ts(g, 128), :])
                xt_tiles.append(xt)

            # ---- recurrence matrix chain prologue (DVE + one transpose) ----
            gmT_ps = psM.tile([T, T], f32, tag="mm")
            nc.tensor.transpose(gmT_ps[:], gm_f[:], ident)
            gmT = constp.tile([T, T], f32, tag="gmT")
            nc.vector.tensor_copy(gmT[:], gmT_ps[:])

            st = {}
            st["Tp"] = mp.tile([T, T], f32, tag="Tp", name="Tp0")
            nc.vector.tensor_tensor(out=st["Tp"][:], in0=gmT[:], in1=msl, op=Alu.mult)
            st["TpT"] = mp.tile([T, T], f32, tag="TpT", name="TpT0")
            nc.vector.tensor_tensor(out=st["TpT"][:], in0=gm_f[:], in1=msu, op=Alu.mult)
            st["G"] = mp.tile([T, T], f32, tag="G", name="G0")
            nc.vector.tensor_tensor(out=st["G"][:], in0=ident, in1=st["Tp"][:], op=Alu.add)
            L_low = constp.tile([T, T], f32, tag="Llow")
            nc.vector.tensor_tensor(out=L_low[:], in0=gm_f[:], in1=mle, op=Alu.mult)
            MT = constp.tile([T, T], f32, tag="MT")

            def chain_step():
                # one squaring of the nilpotent-inverse chain, as 4 drip-ops
                # (PE matmuls; copies/adds on Act so PE never parks on them)
                def mms():
                    st["sq"] = psM.tile([T, T], f32, tag="mm", name="sq")
                    nc.tensor.matmul(st["sq"][:], st["Tp"][:], st["TpT"][:])
                    st["sq2"] = psM.tile([T, T], f32, tag="mm", name="sq2")
                    nc.tensor.matmul(st["sq2"][:], st["TpT"][:], st["Tp"][:])

                def copies():
                    st["Tp"] = mp.tile([T, T], f32, tag="Tp", name="Tpn")
                    nc.vector.tensor_copy(st["Tp"][:], st["sq2"][:])
                    st["TpT"] = mp.tile([T, T], f32, tag="TpT", name="TpTn")
                    nc.vector.tensor_copy(st["TpT"][:], st["sq"][:])

                def mmg():
                    st["gu"] = psM.tile([T, T], f32, tag="mm", name="gu")
                    nc.tensor.matmul(st["gu"][:], st["TpT"][:], st["G"][:])

                def addg():
                    G_n = mp.tile([T, T], f32, tag="G", name="Gn")
                    nc.vector.tensor_tensor(
                        out=G_n[:], in0=st["G"][:], in1=st["gu"][:], op=Alu.add
                    )
                    st["G"] = G_n

                return [mms, copies, mmg, addg]

            chain_ops = []
            for _k in range(6):
                chain_ops.extend(chain_step())
            chain_ops.append(
                lambda: (
                    st.__setitem__("mt", psM.tile([T, T], f32, tag="mm", name="mt")),
                    nc.tensor.matmul(st["mt"][:], L_low[:], st["G"][:]),
                )
            )
            chain_ops.append(lambda: nc.vector.tensor_copy(MT[:], st["mt"][:]))

            loop_cm = (
                tc.For_i(0, loop_stream, 1)
                if loop_stream > 1
                else contextlib.nullcontext()
            )
            def temp_sub(j, s):
                # the (tok, tag) lhsT slice for subtile s of chunk j, from
                # the engine-private tempT tile of engine j % NENG
                return t_eng[j % NENG][:, ts((j // NENG) * SUBS + s, 128)]

            with loop_cm:
                # ---- phase A: temp matmuls + whole-chunk relu round-robin
                #      across DVE/Act/Pool (one reader per ttp tile: multiple
                #      engines on one tile get serialized by the scheduler)
                t_eng = [
                    workp.tile(
                        [128, ((NCHUNK - e + NENG - 1) // NENG) * CHUNK],
                        bf16,
                        tag=f"teng{e}",
                        name=f"t_eng{e}",
                    )
                    for e in range(NENG)
                ]
                for j in range(NCHUNK):
                    g, c = divmod(j, DSPAN)
                    tt_in = tt_tiles[g]
                    ttp = psB.tile([128, CHUNK], f32, tag="tt")
                    for s in range(SUBS):
                        nc.tensor.matmul(
                            ttp[:, ts(s, 128)],
                            tt_in[:, :, ts(c * SUBS + s, 128)],
                            cmat[:],
                            perf_mode=DR,
                        )
                    dst = t_eng[j % NENG][:, ts(j // NENG, CHUNK)]
                    if j % NENG == 0:
                        nc.vector.tensor_scalar_max(dst, ttp[:], 0.0)
                    else:
                        nc.scalar.activation(dst, ttp[:], Relu)

                # ---- phase B: P_aug[tag, :DS] += temp @ x_slice,
                #      P_aug[tag, DS] += rowsum(temp) via the ones column.
                #      The chain drips through phase B's PE/DVE/Pool slack.
                P_ps = psA.tile([128, XW], f32, tag="pacc")
                for g in range(NSPAN_X):
                    xt = xt_tiles[g]
                    for cs in range(DSPAN_X * SUBS):
                        i = g * DSPAN_X * SUBS + cs
                        j, s = divmod(i, SUBS)
                        nc.tensor.matmul(
                            P_ps[:],
                            temp_sub(j, s),
                            xt[:, ts(cs, XW)],
                            start=(i == 0),
                            stop=(i == NSUB - 1),
                        )
                        if i in CHAIN_SCHED and "nochain" not in _VARIANT:
                            chain_ops[CHAIN_SCHED[i]]()
                if "nochain" in _VARIANT:
                    for f in chain_ops:
                        f()

            # ---- out = M @ (diag(1/r) P)  (lhsT = MT) ----
            inv_r = workp.tile([128, 1], f32, tag="invr")
            nc.vector.reciprocal(inv_r[:], P_ps[:, DS : DS + 1])
            P_sb = workp.tile([128, DS], f32, tag="Psb")
            nc.vector.tensor_scalar_mul(P_sb[:], P_ps[:, 0:DS], inv_r[:])

            o_ps = psM.tile([T, T], f32, tag="mm")
            nc.tensor.matmul(o_ps[:], MT[:], P_sb[:])
            out_sb = workp.tile([128, DS], f32, tag="outsb")
            nc.vector.tensor_copy(out_sb[:], o_ps[:])
            nc.sync.dma_start(out_d[:], out_sb[:])

    nc.compile()
    return nc


def _get_program(with_cc=True, loop_stream=1):
    key = (with_cc, loop_stream)
    if key not in _PROGRAM:
        _PROGRAM[key] = _build_program(with_cc, loop_stream)
    return _PROGRAM[key]


def _make_in_maps(inputs):
    import ml_dtypes

    bf16 = ml_dtypes.bfloat16
    x = np.asarray(inputs["inputs"], dtype=np.float32).reshape(N, D)
    t2t = np.asarray(inputs["tag_to_token"], dtype=np.float32).reshape(T, N)
    gm = np.asarray(inputs["gat_mask"], dtype=np.int32).reshape(T, T)
    # DoubleRow layout: [p, i, tok] = binary[i*64 + p, tok]
    t2t_bin = np.ascontiguousarray(
        (t2t > 0).reshape(2, 64, N).transpose(1, 0, 2)
    ).astype(ml_dtypes.float8_e4m3)
    consts = _host_consts()
    in_maps = []
    for c in range(NCORES):
        xc = x[:, c * DS : (c + 1) * DS].astype(bf16)
        xa = np.concatenate([xc, np.ones((N, 1), dtype=bf16)], axis=1)
        # (N, XW) -> span-major layout:
        # [g*128+p, (c*SUBS+s)*XW+q] = xa[((g*DSPAN+c)*SUBS+s)*128+p, q]
        xa = np.ascontiguousarray(
            xa.reshape(NSPAN_X, DSPAN_X * SUBS, 128, XW)
            .transpose(0, 2, 1, 3)
            .reshape(NSPAN_X * 128, DSPAN_X * SUBS * XW)
        )
        m = {"xs": xa, "t2t": t2t_bin, "gm": gm}
        m.update(consts)
        in_maps.append(m)
    return in_maps


def _run(inputs, trace=False, **kw):
    from concourse.bass_utils import run_bass_kernel_spmd

    nc = _get_program()
    in_maps = _make_in_maps(inputs)
    res = run_bass_kernel_spmd(
        nc, in_maps, list(range(NCORES)), trace=trace, **kw
    )
    out = np.empty((T, D), dtype=np.float32)
    for c in range(NCORES):
        out[:, c * DS : (c + 1) * DS] = np.asarray(res.results[c]["out"])
    return out.reshape(B, T, D), res


def kernel(**inputs) -> np.ndarray:
    out, _ = _run(inputs, trace=False)
    return out


# revision 56
# speedup vs baseline: 1.1345x; 1.1345x over previous
"""Bass/Tile TRN2 kernel for nn_Link_83047487635827 (gnn_message_passing).

Math (verified against the reference):
    binary = (tag_to_token > 0)                       # (T, N)
    temp   = relu(C^T @ binary),  C = I - strict_lower_ones(T)
    r      = rowsum(temp); P = temp @ inputs          # (T,), (T, D)
    child  == gat_mask  (reference deduce_child is an identity for 0/1 masks)
    out    = (I - S_up)^{-1} @ L_low @ diag(1/r) @ P
    (I - S_up)^{-1} = prod_{k=0..6} (I + S_up^(2^k))   # S_up nilpotent

Sharding (tensor parallel over D, per the hint): every core loads the FULL
binarized tag_to_token and redundantly computes temp, but only its own
128-column slice of x (bf16) and P.  A ones-column appended to x yields r in
the same PSUM accumulation.  No collective at all; each core writes its
(T, 128) output slice and the host concatenates.

binary and C are 0/+-1, so the temp matmul is EXACT in fp8e4, and with the
DoubleRow perf mode (operands laid out (64, 2, .), contraction 2x64) it runs
at 0.5 PE cycles/row.  temp is 0/1, exact in bf16 for the P matmul; the only
approximation is bf16 rounding of x (~0.15% << the 2e-2 tolerance).

Structure (phase-split to decouple the relu drain from the P matmuls):
  Phase A: stream t2t, temp matmuls, relu+PSUM-drain split across the
           DVE/Act/Pool engines into a RESIDENT (128, N) bf16 tempT buffer.
           The recurrence-matrix chain drips one op per chunk through the
           idle PE/Pool slack.
  Phase B: 256 back-to-back P matmuls accumulating (T, 129) PSUM, gated
           only by the prefetched xs span DMAs.
"""

import numpy as np

B, T, N, D = 1, 128, 32768, 1024
NCORES = 8
DS = D // NCORES          # output columns per core = 128
XW = DS + 1               # x slice + ones column = 129
NSUB = N // 128           # 256 subtiles total
CHUNK = 512               # phase-A tokens per chunk
NCHUNK = N // CHUNK       # 64
SUBS = CHUNK // 128       # 4
DSPAN = 8                 # chunks per t2t DMA span
NSPAN = NCHUNK // DSPAN   # 8 t2t span DMAs
DSPAN_X = 8               # chunks per xs DMA span
NSPAN_X = NCHUNK // DSPAN_X  # 16 xs span DMAs
PSB = 5                   # ttp PSUM bufs
NENG = 2                  # relu engines (DVE/Act: GPSIMD cannot read PSUM)
# chain drip schedule inside phase B: step k ops at P-matmul indices
# base+0 (mms), +8 (copies), +20 (mmg), +28 (addg), stride 36
CHAIN_SCHED = {}
for _k in range(6):
    for _o, _off in enumerate((0, 8, 20, 28)):
        CHAIN_SCHED[_k * 36 + _off] = _k * 4 + _o
CHAIN_SCHED[224] = 24
CHAIN_SCHED[232] = 25

_PROGRAM = {}             # (with_cc, loop_stream) -> nc
_VARIANT = set()          # timing-analysis knobs, empty in production


def _host_consts():
    import ml_dtypes

    f32 = np.float32
    ident = np.eye(T, dtype=f32)
    # C[j, i] = 1 if j == i, -1 if j > i  (temp^T tile = binary_tile^T @ C)
    cmat = np.eye(T, dtype=f32) - np.tril(np.ones((T, T), dtype=f32), -1)
    # DoubleRow layout: [p, i, n] = cmat[i*64 + p, n]
    cmat_dr = np.ascontiguousarray(
        cmat.reshape(2, 64, T).transpose(1, 0, 2)
    ).astype(ml_dtypes.float8_e4m3)
    msl = np.tril(np.ones((T, T), dtype=f32), -1)   # strict lower
    msu = np.triu(np.ones((T, T), dtype=f32), 1)    # strict upper
    mle = np.tril(np.ones((T, T), dtype=f32), 0)    # lower inclusive
    # one DMA for all four (T, T) f32 masks: [ident | msl | msu | mle]
    cpack = np.ascontiguousarray(
        np.concatenate([ident, msl, msu, mle], axis=1)
    )
    return {"cpack": cpack, "cmat": cmat_dr}


def _build_program(with_cc=True, loop_stream=1):
    import contextlib

    import concourse.bacc as bacc
    import concourse.bass as bass
    import concourse.mybir as mybir
    import concourse.tile as tile
    from concourse.bass import ts

    f32 = mybir.dt.float32
    bf16 = mybir.dt.bfloat16
    fp8 = mybir.dt.float8e4
    i32 = mybir.dt.int32
    Alu = mybir.AluOpType
    DR = mybir.MatmulPerfMode.DoubleRow
    Relu = mybir.ActivationFunctionType.Relu
    Copy = mybir.ActivationFunctionType.Copy

    nc = bacc.Bacc(
        "TRN2", target_bir_lowering=False, debug=False, num_devices=NCORES
    )

    # x slice, host-permuted by DMA span: row g*128+p, col (c*SUBS+s)*XW+q =
    # x_aug[((g*DSPAN_X+c)*SUBS+s)*128+p, q]
    xs_d = nc.dram_tensor(
        "xs", (NSPAN_X * 128, DSPAN_X * SUBS * XW), bf16, kind="ExternalInput"
    )
    # DoubleRow layout: [p, i, tok] = binary[i*64 + p, tok]
    t2t_d = nc.dram_tensor("t2t", (64, 2, N), fp8, kind="ExternalInput")
    gm_d = nc.dram_tensor("gm", (T, T), i32, kind="ExternalInput")
    # packed f32 masks: [ident | msl | msu | mle]
    cpack_d = nc.dram_tensor("cpack", (T, 4 * T), f32, kind="ExternalInput")
    cmat_d = nc.dram_tensor("cmat", (64, 2, T), fp8, kind="ExternalInput")
    out_d = nc.dram_tensor("out", (T, DS), f32, kind="ExternalOutput")

    with tile.TileContext(nc) as tc:
        with (
            tc.tile_pool(name="const", bufs=1) as constp,
            tc.tile_pool(name="xin", bufs=NSPAN_X) as xp,
            tc.tile_pool(name="t2tin", bufs=NSPAN) as t2tp,
            tc.tile_pool(name="work", bufs=1) as workp,
            tc.tile_pool(name="mchain", bufs=2) as mp,
            tc.tile_pool(name="psacc", bufs=1, space=bass.MemorySpace.PSUM) as psA,
            tc.tile_pool(name="pstt", bufs=PSB, space=bass.MemorySpace.PSUM) as psB,
            tc.tile_pool(name="psm", bufs=2, space=bass.MemorySpace.PSUM) as psM,
        ):
            # ---- DMAs: cmat first (gates the first temp matmul), then the
            #      t2t spans (gate phase A), xs spans, small consts last ----
            cmat = constp.tile([64, 2, T], fp8, tag="cmat")
            nc.sync.dma_start(cmat[:], cmat_d[:])
            tt_tiles = [
                t2tp.tile([64, 2, DSPAN * CHUNK], fp8, tag="ttin", name=f"tt{g}")
                for g in range(NSPAN)
            ]
            xt_tiles = [
                xp.tile([128, DSPAN_X * SUBS * XW], bf16, tag="xt", name=f"xt{g}")
                for g in range(NSPAN_X)
            ]
            cpk = constp.tile([T, 4 * T], f32, tag="cpack")
            gm_i = constp.tile([T, T], i32, tag="gmi")

            def dma_t(g):
                nc.sync.dma_start(tt_tiles[g][:], t2t_d[:, :, ts(g, DSPAN * CHUNK)])

            def dma_x(g):
                nc.sync.dma_start(xt_tiles[g][:], xs_d[ts(g, 128), :])

            if "nodma" not in _VARIANT:
                dma_t(0)
                dma_t(1)
                dma_x(0)
                nc.sync.dma_start(cpk[:], cpack_d[:])
                nc.sync.dma_start(gm_i[:], gm_d[:])
                for g in range(2, NSPAN):
                    dma_t(g)
                    dma_x(g - 1)
                dma_x(NSPAN_X - 1)
            ident, msl, msu, mle = (cpk[:, ts(k, T)] for k in range(4))
            gm_f = constp.tile([T, T], f32, tag="gmf")
            nc.vector.tensor_copy(gm_f[:], gm_i[:])

            # ---- recurrence matrix chain prologue (DVE + one transpose) ----
            gmT_ps = psM.tile([T, T], f32, tag="mm")
            nc.tensor.transpose(gmT_ps[:], gm_f[:], ident)
            gmT = constp.tile([T, T], f32, tag="gmT")
            nc.vector.tensor_copy(gmT[:], gmT_ps[:])

            st = {}
            st["Tp"] = mp.tile([T, T], f32, tag="Tp", name="Tp0")
            nc.vector.tensor_tensor(out=st["Tp"][:], in0=gmT[:], in1=msl, op=Alu.mult)
            st["TpT"] = mp.tile([T, T], f32, tag="TpT", name="TpT0")
            nc.vector.tensor_tensor(out=st["TpT"][:], in0=gm_f[:], in1=msu, op=Alu.mult)
            st["G"] = mp.tile([T, T], f32, tag="G", name="G0")
            nc.vector.tensor_tensor(out=st["G"][:], in0=ident, in1=st["Tp"][:], op=Alu.add)
            L_low = constp.tile([T, T], f32, tag="Llow")
            nc.vector.tensor_tensor(out=L_low[:], in0=gm_f[:], in1=mle, op=Alu.mult)
            MT = constp.tile([T, T], f32, tag="MT")

            def chain_step():
                # one squaring of the nilpotent-inverse chain, as 4 drip-ops
                # (PE matmuls; copies/adds on Act so PE never parks on them)
                def mms():
                    st["sq"] = psM.tile([T, T], f32, tag="mm", name="sq")
                    nc.tensor.matmul(st["sq"][:], st["Tp"][:], st["TpT"][:])
                    st["sq2"] = psM.tile([T, T], f32, tag="mm", name="sq2")
                    nc.tensor.matmul(st["sq2"][:], st["TpT"][:], st["Tp"][:])

                def copies():
                    st["Tp"] = mp.tile([T, T], f32, tag="Tp", name="Tpn")
                    nc.scalar.activation(st["Tp"][:], st["sq2"][:], Copy)
                    st["TpT"] = mp.tile([T, T], f32, tag="TpT", name="TpTn")
                    nc.scalar.activation(st["TpT"][:], st["sq"][:], Copy)

                def mmg():
                    st["gu"] = psM.tile([T, T], f32, tag="mm", name="gu")
                    nc.tensor.matmul(st["gu"][:], st["TpT"][:], st["G"][:])

                def addg():
                    G_n = mp.tile([T, T], f32, tag="G", name="Gn")
                    nc.vector.tensor_tensor(
                        out=G_n[:], in0=st["G"][:], in1=st["gu"][:], op=Alu.add
                    )
                    st["G"] = G_n

                return [mms, copies, mmg, addg]

            chain_ops = []
            for _k in range(6):
                chain_ops.extend(chain_step())
            chain_ops.append(
                lambda: (
                    st.__setitem__("mt", psM.tile([T, T], f32, tag="mm", name="mt")),
                    nc.tensor.matmul(st["mt"][:], L_low[:], st["G"][:]),
                )
            )
            chain_ops.append(lambda: nc.scalar.activation(MT[:], st["mt"][:], Copy))

            loop_cm = (
                tc.For_i(0, loop_stream, 1)
                if loop_stream > 1
                else contextlib.nullcontext()
            )
            def temp_sub(j, s):
                # the (tok, tag) lhsT slice for subtile s of chunk j, from
                # the engine-private tempT tile of engine j % NENG
                return t_eng[j % NENG][:, ts((j // NENG) * SUBS + s, 128)]

            with loop_cm:
                # ---- span-interleaved: temps+relus for span g, then the
                #      P matmuls for span g-1 (a full span of slack decouples
                #      the PE from the relu drain) ----
                t_eng = [
                    workp.tile(
                        [128, ((NCHUNK - e + NENG - 1) // NENG) * CHUNK],
                        bf16,
                        tag=f"teng{e}",
                        name=f"t_eng{e}",
                    )
                    for e in range(NENG)
                ]
                P_ps = psA.tile([128, XW], f32, tag="pacc")

                def emit_span_P(g):
                    xt = xt_tiles[g]
                    for cs in range(DSPAN_X * SUBS):
                        i = g * DSPAN_X * SUBS + cs
                        j, s = divmod(i, SUBS)
                        nc.tensor.matmul(
                            P_ps[:],
                            temp_sub(j, s),
                            xt[:, ts(cs, XW)],
                            start=(i == 0),
                            stop=(i == NSUB - 1),
                        )

                for g in range(NSPAN):
                    for c in range(DSPAN):
                        j = g * DSPAN + c
                        tt_in = tt_tiles[g]
                        ttp = psB.tile([128, CHUNK], f32, tag="tt")
                        for s in range(SUBS):
                            nc.tensor.matmul(
                                ttp[:, ts(s, 128)],
                                tt_in[:, :, ts(c * SUBS + s, 128)],
                                cmat[:],
                                perf_mode=DR,
                            )
                        dst = t_eng[j % NENG][:, ts(j // NENG, CHUNK)]
                        if j % NENG == 0:
                            nc.vector.tensor_scalar_max(dst, ttp[:], 0.0)
                        else:
                            nc.scalar.activation(dst, ttp[:], Relu)
                        if (
                            j >= 2
                            and j % 2 == 0
                            and (j - 2) // 2 < len(chain_ops)
                            and "nochain" not in _VARIANT
                        ):
                            chain_ops[(j - 2) // 2]()
                    if g >= 1:
                        emit_span_P(g - 1)
                emit_span_P(NSPAN - 1)
                if "nochain" in _VARIANT:
                    for f in chain_ops:
                        f()

            # ---- out = M @ (diag(1/r) P)  (lhsT = MT) ----
            inv_r = workp.tile([128, 1], f32, tag="invr")
            nc.vector.reciprocal(inv_r[:], P_ps[:, DS : DS + 1])
            P_sb = workp.tile([128, DS], f32, tag="Psb")
            nc.vector.tensor_scalar_mul(P_sb[:], P_ps[:, 0:DS], inv_r[:])

            o_ps = psM.tile([T, T], f32, tag="mm")
            nc.tensor.matmul(o_ps[:], MT[:], P_sb[:])
            out_sb = workp.tile([128, DS], f32, tag="outsb")
            nc.vector.tensor_copy(out_sb[:], o_ps[:])
            nc.sync.dma_start(out_d[:], out_sb[:])

    nc.compile()
    return nc


def _get_program(with_cc=True, loop_stream=1):
    key = (with_cc, loop_stream)
    if key not in _PROGRAM:
        _PROGRAM[key] = _build_program(with_cc, loop_stream)
    return _PROGRAM[key]


def _make_in_maps(inputs):
    import ml_dtypes

    bf16 = ml_dtypes.bfloat16
    x = np.asarray(inputs["inputs"], dtype=np.float32).reshape(N, D)
    t2t = np.asarray(inputs["tag_to_token"], dtype=np.float32).reshape(T, N)
    gm = np.asarray(inputs["gat_mask"], dtype=np.int32).reshape(T, T)
    # DoubleRow layout: [p, i, tok] = binary[i*64 + p, tok]
    t2t_bin = np.ascontiguousarray(
        (t2t > 0).reshape(2, 64, N).transpose(1, 0, 2)
    ).astype(ml_dtypes.float8_e4m3)
    consts = _host_consts()
    in_maps = []
    for c in range(NCORES):
        xc = x[:, c * DS : (c + 1) * DS].astype(bf16)
        xa = np.concatenate([xc, np.ones((N, 1), dtype=bf16)], axis=1)
        # (N, XW) -> span-major layout:
        # [g*128+p, (c*SUBS+s)*XW+q] = xa[((g*DSPAN+c)*SUBS+s)*128+p, q]
        xa = np.ascontiguousarray(
            xa.reshape(NSPAN_X, DSPAN_X * SUBS, 128, XW)
            .transpose(0, 2, 1, 3)
            .reshape(NSPAN_X * 128, DSPAN_X * SUBS * XW)
        )
        m = {"xs": xa, "t2t": t2t_bin, "gm": gm}
        m.update(consts)
        in_maps.append(m)
    return in_maps


def _run(inputs, trace=False, **kw):
    from concourse.bass_utils import run_bass_kernel_spmd

    nc = _get_program()
    in_maps = _make_in_maps(inputs)
    res = run_bass_kernel_spmd(
        nc, in_maps, list(range(NCORES)), trace=trace, **kw
    )
    out = np.empty((T, D), dtype=np.float32)
    for c in range(NCORES):
        out[:, c * DS : (c + 1) * DS] = np.asarray(res.results[c]["out"])
    return out.reshape(B, T, D), res


def kernel(**inputs) -> np.ndarray:
    out, _ = _run(inputs, trace=False)
    return out


# revision 58
# speedup vs baseline: 1.1482x; 1.0121x over previous
"""Bass/Tile TRN2 kernel for nn_Link_83047487635827 (gnn_message_passing).

Math (verified against the reference):
    binary = (tag_to_token > 0)                       # (T, N)
    temp   = relu(C^T @ binary),  C = I - strict_lower_ones(T)
    r      = rowsum(temp); P = temp @ inputs          # (T,), (T, D)
    child  == gat_mask  (reference deduce_child is an identity for 0/1 masks)
    out    = (I - S_up)^{-1} @ L_low @ diag(1/r) @ P
    (I - S_up)^{-1} = prod_{k=0..6} (I + S_up^(2^k))   # S_up nilpotent

Sharding (tensor parallel over D, per the hint): every core loads the FULL
binarized tag_to_token and redundantly computes temp, but only its own
128-column slice of x (bf16) and P.  A ones-column appended to x yields r in
the same PSUM accumulation.  No collective at all; each core writes its
(T, 128) output slice and the host concatenates.

binary and C are 0/+-1, so the temp matmul is EXACT in fp8e4, and with the
DoubleRow perf mode (operands laid out (64, 2, .), contraction 2x64) it runs
at 0.5 PE cycles/row.  temp is 0/1, exact in bf16 for the P matmul; the only
approximation is bf16 rounding of x (~0.15% << the 2e-2 tolerance).

Structure (span-interleaved; every input DMA is issued up front and all
tiles stay resident, so nothing ever waits on a buffer):
  span g: temp matmuls + whole-chunk relu drains round-robin on DVE/Act
          (one consumer per PSUM tile: multiple engines on one tile get
          serialized by the scheduler) into resident tempT, then the
          P matmuls of span g-1 — a full span of slack decouples the
          in-order PE from the relu latency.  The recurrence-matrix chain
          drips one op per even chunk through the early spans' PE slack.
"""

import numpy as np

B, T, N, D = 1, 128, 32768, 1024
NCORES = 8
DS = D // NCORES          # output columns per core = 128
XW = DS + 1               # x slice + ones column = 129
NSUB = N // 128           # 256 subtiles total
CHUNK = 512               # phase-A tokens per chunk
NCHUNK = N // CHUNK       # 64
SUBS = CHUNK // 128       # 4
DSPAN = 8                 # chunks per t2t DMA span
NSPAN = NCHUNK // DSPAN   # 8 t2t span DMAs
DSPAN_X = 8               # chunks per xs DMA span
NSPAN_X = NCHUNK // DSPAN_X  # 16 xs span DMAs
PSB = 5                   # ttp PSUM bufs
NENG = 2                  # relu engines (DVE/Act: GPSIMD cannot read PSUM)
_PROGRAM = {}             # (with_cc, loop_stream) -> nc
_VARIANT = set()          # timing-analysis knobs, empty in production


def _host_consts():
    import ml_dtypes

    f32 = np.float32
    ident = np.eye(T, dtype=f32)
    # C[j, i] = 1 if j == i, -1 if j > i  (temp^T tile = binary_tile^T @ C)
    cmat = np.eye(T, dtype=f32) - np.tril(np.ones((T, T), dtype=f32), -1)
    # DoubleRow layout: [p, i, n] = cmat[i*64 + p, n]
    cmat_dr = np.ascontiguousarray(
        cmat.reshape(2, 64, T).transpose(1, 0, 2)
    ).astype(ml_dtypes.float8_e4m3)
    msl = np.tril(np.ones((T, T), dtype=f32), -1)   # strict lower
    msu = np.triu(np.ones((T, T), dtype=f32), 1)    # strict upper
    mle = np.tril(np.ones((T, T), dtype=f32), 0)    # lower inclusive
    # one DMA for all four (T, T) f32 masks: [ident | msl | msu | mle]
    cpack = np.ascontiguousarray(
        np.concatenate([ident, msl, msu, mle], axis=1)
    )
    return {"cpack": cpack, "cmat": cmat_dr}


def _build_program(with_cc=True, loop_stream=1):
    import contextlib

    import concourse.bacc as bacc
    import concourse.bass as bass
    import concourse.mybir as mybir
    import concourse.tile as tile
    from concourse.bass import ts

    f32 = mybir.dt.float32
    bf16 = mybir.dt.bfloat16
    fp8 = mybir.dt.float8e4
    i32 = mybir.dt.int32
    Alu = mybir.AluOpType
    DR = mybir.MatmulPerfMode.DoubleRow
    Relu = mybir.ActivationFunctionType.Relu
    Copy = mybir.ActivationFunctionType.Copy

    nc = bacc.Bacc(
        "TRN2", target_bir_lowering=False, debug=False, num_devices=NCORES
    )

    # x slice, host-permuted by DMA span: row g*128+p, col (c*SUBS+s)*XW+q =
    # x_aug[((g*DSPAN_X+c)*SUBS+s)*128+p, q]
    xs_d = nc.dram_tensor(
        "xs", (NSPAN_X * 128, DSPAN_X * SUBS * XW), bf16, kind="ExternalInput"
    )
    # DoubleRow layout: [p, i, tok] = binary[i*64 + p, tok]
    t2t_d = nc.dram_tensor("t2t", (64, 2, N), fp8, kind="ExternalInput")
    gm_d = nc.dram_tensor("gm", (T, T), i32, kind="ExternalInput")
    # packed f32 masks: [ident | msl | msu | mle]
    cpack_d = nc.dram_tensor("cpack", (T, 4 * T), f32, kind="ExternalInput")
    cmat_d = nc.dram_tensor("cmat", (64, 2, T), fp8, kind="ExternalInput")
    out_d = nc.dram_tensor("out", (T, DS), f32, kind="ExternalOutput")

    with tile.TileContext(nc) as tc:
        with (
            tc.tile_pool(name="const", bufs=1) as constp,
            tc.tile_pool(name="xin", bufs=NSPAN_X) as xp,
            tc.tile_pool(name="t2tin", bufs=NSPAN) as t2tp,
            tc.tile_pool(name="work", bufs=1) as workp,
            tc.tile_pool(name="mchain", bufs=2) as mp,
            tc.tile_pool(name="psacc", bufs=1, space=bass.MemorySpace.PSUM) as psA,
            tc.tile_pool(name="pstt", bufs=PSB, space=bass.MemorySpace.PSUM) as psB,
            tc.tile_pool(name="psm", bufs=2, space=bass.MemorySpace.PSUM) as psM,
        ):
            # ---- DMAs: cmat first (gates the first temp matmul), then the
            #      t2t spans (gate phase A), xs spans, small consts last ----
            cmat = constp.tile([64, 2, T], fp8, tag="cmat")
            nc.sync.dma_start(cmat[:], cmat_d[:])
            tt_tiles = [
                t2tp.tile([64, 2, DSPAN * CHUNK], fp8, tag="ttin", name=f"tt{g}")
                for g in range(NSPAN)
            ]
            xt_tiles = [
                xp.tile([128, DSPAN_X * SUBS * XW], bf16, tag="xt", name=f"xt{g}")
                for g in range(NSPAN_X)
            ]
            cpk = constp.tile([T, 4 * T], f32, tag="cpack")
            gm_i = constp.tile([T, T], i32, tag="gmi")

            def dma_t(g):
                nc.sync.dma_start(tt_tiles[g][:], t2t_d[:, :, ts(g, DSPAN * CHUNK)])

            def dma_x(g):
                nc.sync.dma_start(xt_tiles[g][:], xs_d[ts(g, 128), :])

            if "nodma" not in _VARIANT:
                dma_t(0)
                dma_t(1)
                dma_x(0)
                nc.sync.dma_start(cpk[:], cpack_d[:])
                nc.sync.dma_start(gm_i[:], gm_d[:])
                for g in range(2, NSPAN):
                    dma_t(g)
                    dma_x(g - 1)
                # last span in two halves so its P matmuls start sooner
                half = DSPAN_X * SUBS * XW // 2
                lg = NSPAN_X - 1
                nc.sync.dma_start(
                    xt_tiles[lg][:, 0:half], xs_d[ts(lg, 128), 0:half]
                )
                nc.sync.dma_start(
                    xt_tiles[lg][:, half:], xs_d[ts(lg, 128), half:]
                )
            ident, msl, msu, mle = (cpk[:, ts(k, T)] for k in range(4))
            gm_f = constp.tile([T, T], f32, tag="gmf")
            nc.vector.tensor_copy(gm_f[:], gm_i[:])

            # ---- recurrence matrix chain prologue (DVE + one transpose) ----
            gmT_ps = psM.tile([T, T], f32, tag="mm")
            nc.tensor.transpose(gmT_ps[:], gm_f[:], ident)
            gmT = constp.tile([T, T], f32, tag="gmT")
            nc.vector.tensor_copy(gmT[:], gmT_ps[:])

            st = {}
            st["Tp"] = mp.tile([T, T], f32, tag="Tp", name="Tp0")
            nc.vector.tensor_tensor(out=st["Tp"][:], in0=gmT[:], in1=msl, op=Alu.mult)
            st["TpT"] = mp.tile([T, T], f32, tag="TpT", name="TpT0")
            nc.vector.tensor_tensor(out=st["TpT"][:], in0=gm_f[:], in1=msu, op=Alu.mult)
            st["G"] = mp.tile([T, T], f32, tag="G", name="G0")
            nc.vector.tensor_tensor(out=st["G"][:], in0=ident, in1=st["Tp"][:], op=Alu.add)
            L_low = constp.tile([T, T], f32, tag="Llow")
            nc.vector.tensor_tensor(out=L_low[:], in0=gm_f[:], in1=mle, op=Alu.mult)
            MT = constp.tile([T, T], f32, tag="MT")

            def chain_step():
                # one squaring of the nilpotent-inverse chain, as 4 drip-ops
                # (PE matmuls; copies/adds on Act so PE never parks on them)
                def mms():
                    st["sq"] = psM.tile([T, T], f32, tag="mm", name="sq")
                    nc.tensor.matmul(st["sq"][:], st["Tp"][:], st["TpT"][:])
                    st["sq2"] = psM.tile([T, T], f32, tag="mm", name="sq2")
                    nc.tensor.matmul(st["sq2"][:], st["TpT"][:], st["Tp"][:])

                def copies():
                    st["Tp"] = mp.tile([T, T], f32, tag="Tp", name="Tpn")
                    nc.scalar.activation(st["Tp"][:], st["sq2"][:], Copy)
                    st["TpT"] = mp.tile([T, T], f32, tag="TpT", name="TpTn")
                    nc.scalar.activation(st["TpT"][:], st["sq"][:], Copy)

                def mmg():
                    st["gu"] = psM.tile([T, T], f32, tag="mm", name="gu")
                    nc.tensor.matmul(st["gu"][:], st["TpT"][:], st["G"][:])

                def addg():
                    G_n = mp.tile([T, T], f32, tag="G", name="Gn")
                    nc.vector.tensor_tensor(
                        out=G_n[:], in0=st["G"][:], in1=st["gu"][:], op=Alu.add
                    )
                    st["G"] = G_n

                return [mms, copies, mmg, addg]

            chain_ops = []
            for _k in range(6):
                chain_ops.extend(chain_step())
            chain_ops.append(
                lambda: (
                    st.__setitem__("mt", psM.tile([T, T], f32, tag="mm", name="mt")),
                    nc.tensor.matmul(st["mt"][:], L_low[:], st["G"][:]),
                )
            )
            chain_ops.append(lambda: nc.scalar.activation(MT[:], st["mt"][:], Copy))

            loop_cm = (
                tc.For_i(0, loop_stream, 1)
                if loop_stream > 1
                else contextlib.nullcontext()
            )
            def temp_sub(j, s):
                # the (tok, tag) lhsT slice for subtile s of chunk j, from
                # the engine-private tempT tile of engine j % NENG
                return t_eng[j % NENG][:, ts((j // NENG) * SUBS + s, 128)]

            with loop_cm:
                # ---- span-interleaved: temps+relus for span g, then the
                #      P matmuls for span g-1 (a full span of slack decouples
                #      the PE from the relu drain) ----
                t_eng = [
                    workp.tile(
                        [128, ((NCHUNK - e + NENG - 1) // NENG) * CHUNK],
                        bf16,
                        tag=f"teng{e}",
                        name=f"t_eng{e}",
                    )
                    for e in range(NENG)
                ]
                P_ps = psA.tile([128, XW], f32, tag="pacc")

                def emit_span_P(g):
                    xt = xt_tiles[g]
                    for cs in range(DSPAN_X * SUBS):
                        i = g * DSPAN_X * SUBS + cs
                        j, s = divmod(i, SUBS)
                        nc.tensor.matmul(
                            P_ps[:],
                            temp_sub(j, s),
                            xt[:, ts(cs, XW)],
                            start=(i == 0),
                            stop=(i == NSUB - 1),
                        )

                for g in range(NSPAN):
                    for c in range(DSPAN):
                        j = g * DSPAN + c
                        tt_in = tt_tiles[g]
                        ttp = psB.tile([128, CHUNK], f32, tag="tt")
                        for s in range(SUBS):
                            nc.tensor.matmul(
                                ttp[:, ts(s, 128)],
                                tt_in[:, :, ts(c * SUBS + s, 128)],
                                cmat[:],
                                perf_mode=DR,
                            )
                        dst = t_eng[j % NENG][:, ts(j // NENG, CHUNK)]
                        if j % NENG == 0:
                            nc.vector.tensor_scalar_max(dst, ttp[:], 0.0)
                        else:
                            nc.scalar.activation(dst, ttp[:], Relu)
                        if (
                            j >= 2
                            and j % 2 == 0
                            and (j - 2) // 2 < len(chain_ops)
                            and "nochain" not in _VARIANT
                        ):
                            chain_ops[(j - 2) // 2]()
                    if g >= 1:
                        emit_span_P(g - 1)
                emit_span_P(NSPAN - 1)
                if "nochain" in _VARIANT:
                    for f in chain_ops:
                        f()

            # ---- out = M @ (diag(1/r) P)  (lhsT = MT) ----
            inv_r = workp.tile([128, 1], f32, tag="invr")
            nc.vector.reciprocal(inv_r[:], P_ps[:, DS : DS + 1])
            P_sb = workp.tile([128, DS], f32, tag="Psb")
            nc.vector.tensor_scalar_mul(P_sb[:], P_ps[:, 0:DS], inv_r[:])

            o_ps = psM.tile([T, T], f32, tag="mm")
            nc.tensor.matmul(o_ps[:], MT[:], P_sb[:])
            out_sb = workp.tile([128, DS], f32, tag="outsb")
            nc.vector.tensor_copy(out_sb[:], o_ps[:])
            nc.sync.dma_start(out_d[:], out_sb[:])

    nc.compile()
    return nc


def _get_program(with_cc=True, loop_stream=1):
    key = (with_cc, loop_stream)
    if key not in _PROGRAM:
        _PROGRAM[key] = _build_program(with_cc, loop_stream)
    return _PROGRAM[key]


def _make_in_maps(inputs):
    import ml_dtypes

    bf16 = ml_dtypes.bfloat16
    x = np.asarray(inputs["inputs"], dtype=np.float32).reshape(N, D)
    t2t = np.asarray(inputs["tag_to_token"], dtype=np.float32).reshape(T, N)
    gm = np.asarray(inputs["gat_mask"], dtype=np.int32).reshape(T, T)
    # DoubleRow layout: [p, i, tok] = binary[i*64 + p, tok]
    t2t_bin = np.ascontiguousarray(
        (t2t > 0).reshape(2, 64, N).transpose(1, 0, 2)
    ).astype(ml_dtypes.float8_e4m3)
    consts = _host_consts()
    in_maps = []
    for c in range(NCORES):
        xc = x[:, c * DS : (c + 1) * DS].astype(bf16)
        xa = np.concatenate([xc, np.ones((N, 1), dtype=bf16)], axis=1)
        # (N, XW) -> span-major layout:
        # [g*128+p, (c*SUBS+s)*XW+q] = xa[((g*DSPAN+c)*SUBS+s)*128+p, q]
        xa = np.ascontiguousarray(
            xa.reshape(NSPAN_X, DSPAN_X * SUBS, 128, XW)
            .transpose(0, 2, 1, 3)
            .reshape(NSPAN_X * 128, DSPAN_X * SUBS * XW)
        )
        m = {"xs": xa, "t2t": t2t_bin, "gm": gm}
        m.update(consts)
        in_maps.append(m)
    return in_maps


def _run(inputs, trace=False, **kw):
    from concourse.bass_utils import run_bass_kernel_spmd

    nc = _get_program()
    in_maps = _make_in_maps(inputs)
    res = run_bass_kernel_spmd(
        nc, in_maps, list(range(NCORES)), trace=trace, **kw
    )
    out = np.empty((T, D), dtype=np.float32)
    for c in range(NCORES):
        out[:, c * DS : (c + 1) * DS] = np.asarray(res.results[c]["out"])
    return out.reshape(B, T, D), res


def kernel(**inputs) -> np.ndarray:
    out, _ = _run(inputs, trace=False)
    return out


# revision 60
# speedup vs baseline: 1.3187x; 1.1485x over previous
"""Bass/Tile TRN2 kernel for nn_Link_83047487635827 (gnn_message_passing).

Math (verified against the reference):
    binary = (tag_to_token > 0)                       # (T, N)
    temp   = relu(C^T @ binary),  C = I - strict_lower_ones(T)
    r      = rowsum(temp); P = temp @ inputs          # (T,), (T, D)
    child  == gat_mask  (reference deduce_child is an identity for 0/1 masks)
    out    = (I - S_up)^{-1} @ L_low @ diag(1/r) @ P
    (I - S_up)^{-1} = prod_{k=0..6} (I + S_up^(2^k))   # S_up nilpotent

Sharding (tensor parallel over D, per the hint): every core loads the FULL
binarized tag_to_token and redundantly computes temp, but only its own
128-column slice of x (bf16) and P.  A ones-column appended to x yields r in
the same PSUM accumulation.  No collective at all; each core writes its
(T, 128) output slice and the host concatenates.

binary and C are 0/+-1, so the temp matmul is EXACT in fp8e4, and with the
DoubleRow perf mode (operands laid out (64, 2, .), contraction 2x64) it runs
at 0.5 PE cycles/row.  temp is 0/1, exact in bf16 for the P matmul; the only
approximation is bf16 rounding of x (~0.15% << the 2e-2 tolerance).

Structure (span-interleaved; every input DMA is issued up front and all
tiles stay resident, so nothing ever waits on a buffer):
  span g: temp matmuls + whole-chunk relu drains round-robin on DVE/Act
          (one consumer per PSUM tile: multiple engines on one tile get
          serialized by the scheduler) into resident tempT, then the
          P matmuls of span g-1 — a full span of slack decouples the
          in-order PE from the relu latency.  The recurrence-matrix chain
          drips one op per even chunk through the early spans' PE slack.
"""

import numpy as np

B, T, N, D = 1, 128, 32768, 1024
NCORES = 8
DS = D // NCORES          # output columns per core = 128
XW = DS + 1               # x slice + ones column = 129
NSUB = N // 128           # 256 subtiles total
CHUNK = 512               # phase-A tokens per chunk
NCHUNK = N // CHUNK       # 64
SUBS = CHUNK // 128       # 4
DSPAN = 8                 # chunks per t2t DMA span
NSPAN = NCHUNK // DSPAN   # 8 t2t span DMAs
DSPAN_X = 8               # chunks per xs DMA span
NSPAN_X = NCHUNK // DSPAN_X  # 16 xs span DMAs
PSB = 5                   # ttp PSUM bufs
NENG = 2                  # relu engines (DVE/Act: GPSIMD cannot read PSUM)
_PROGRAM = {}             # (with_cc, loop_stream) -> nc
_VARIANT = set()          # timing-analysis knobs, empty in production


def _host_consts():
    import ml_dtypes

    f32 = np.float32
    ident = np.eye(T, dtype=f32)
    # C[j, i] = 1 if j == i, -1 if j > i  (temp^T tile = binary_tile^T @ C)
    cmat = np.eye(T, dtype=f32) - np.tril(np.ones((T, T), dtype=f32), -1)
    # DoubleRow layout: [p, i, n] = cmat[i*64 + p, n]
    cmat_dr = np.ascontiguousarray(
        cmat.reshape(2, 64, T).transpose(1, 0, 2)
    ).astype(ml_dtypes.float8_e4m3)
    msl = np.tril(np.ones((T, T), dtype=f32), -1)   # strict lower
    msu = np.triu(np.ones((T, T), dtype=f32), 1)    # strict upper
    mle = np.tril(np.ones((T, T), dtype=f32), 0)    # lower inclusive
    # one DMA for all four (T, T) f32 masks: [ident | msl | msu | mle]
    cpack = np.ascontiguousarray(
        np.concatenate([ident, msl, msu, mle], axis=1)
    )
    return {"cpack": cpack, "cmat": cmat_dr}


def _build_program(with_cc=True, loop_stream=1):
    import contextlib

    import concourse.bacc as bacc
    import concourse.bass as bass
    import concourse.mybir as mybir
    import concourse.tile as tile
    from concourse.bass import ts

    f32 = mybir.dt.float32
    bf16 = mybir.dt.bfloat16
    fp8 = mybir.dt.float8e4
    i32 = mybir.dt.int32
    Alu = mybir.AluOpType
    DR = mybir.MatmulPerfMode.DoubleRow
    Relu = mybir.ActivationFunctionType.Relu
    Copy = mybir.ActivationFunctionType.Copy

    nc = bacc.Bacc(
        "TRN2", target_bir_lowering=False, debug=False, num_devices=NCORES
    )

    # x slice, host-permuted by DMA span: row g*128+p, col (c*SUBS+s)*XW+q =
    # x_aug[((g*DSPAN_X+c)*SUBS+s)*128+p, q]
    xs_d = nc.dram_tensor(
        "xs", (NSPAN_X * 128, DSPAN_X * SUBS * XW), bf16, kind="ExternalInput"
    )
    # DoubleRow layout: [p, i, tok] = binary[i*64 + p, tok]
    t2t_d = nc.dram_tensor("t2t", (64, 2, N), fp8, kind="ExternalInput")
    gm_d = nc.dram_tensor("gm", (T, T), i32, kind="ExternalInput")
    # packed f32 masks: [ident | msl | msu | mle]
    cpack_d = nc.dram_tensor("cpack", (T, 4 * T), f32, kind="ExternalInput")
    cmat_d = nc.dram_tensor("cmat", (64, 2, T), fp8, kind="ExternalInput")
    out_d = nc.dram_tensor("out", (T, DS), f32, kind="ExternalOutput")

    with tile.TileContext(nc) as tc:
        with (
            tc.tile_pool(name="const", bufs=1) as constp,
            tc.tile_pool(name="xin", bufs=NSPAN_X) as xp,
            tc.tile_pool(name="t2tin", bufs=NSPAN) as t2tp,
            tc.tile_pool(name="work", bufs=1) as workp,
            tc.tile_pool(name="mchain", bufs=2) as mp,
            tc.tile_pool(name="psacc", bufs=1, space=bass.MemorySpace.PSUM) as psA,
            tc.tile_pool(name="pstt", bufs=PSB, space=bass.MemorySpace.PSUM) as psB,
            tc.tile_pool(name="psm", bufs=2, space=bass.MemorySpace.PSUM) as psM,
        ):
            # ---- DMAs: cmat first (gates the first temp matmul), then the
            #      t2t spans (gate phase A), xs spans, small consts last ----
            cmat = constp.tile([64, 2, T], fp8, tag="cmat")
            nc.sync.dma_start(cmat[:], cmat_d[:])
            tt_tiles = [
                t2tp.tile([64, 2, DSPAN * CHUNK], fp8, tag="ttin", name=f"tt{g}")
                for g in range(NSPAN)
            ]
            xt_tiles = [
                xp.tile([128, DSPAN_X * SUBS * XW], bf16, tag="xt", name=f"xt{g}")
                for g in range(NSPAN_X)
            ]
            cpk = constp.tile([T, 4 * T], f32, tag="cpack")
            gm_i = constp.tile([T, T], i32, tag="gmi")

            def dma_t(g):
                nc.sync.dma_start(tt_tiles[g][:], t2t_d[:, :, ts(g, DSPAN * CHUNK)])

            def dma_x(g):
                nc.sync.dma_start(xt_tiles[g][:], xs_d[ts(g, 128), :])

            if "nodma" not in _VARIANT:
                dma_t(0)
                dma_t(1)
                dma_t(2)
                dma_x(0)
                nc.sync.dma_start(cpk[:], cpack_d[:])
                nc.sync.dma_start(gm_i[:], gm_d[:])
                for g in range(3, NSPAN):
                    dma_t(g)
                    dma_x(g - 2)
                for g in range(NSPAN - 2, NSPAN_X):
                    dma_x(g)
            ident, msl, msu, mle = (cpk[:, ts(k, T)] for k in range(4))
            gm_f = constp.tile([T, T], f32, tag="gmf")
            nc.vector.tensor_copy(gm_f[:], gm_i[:])

            # ---- recurrence matrix chain prologue (DVE + one transpose) ----
            gmT_ps = psM.tile([T, T], f32, tag="mm")
            nc.tensor.transpose(gmT_ps[:], gm_f[:], ident)
            gmT = constp.tile([T, T], f32, tag="gmT")
            nc.vector.tensor_copy(gmT[:], gmT_ps[:])

            st = {}
            st["Tp"] = mp.tile([T, T], f32, tag="Tp", name="Tp0")
            nc.vector.tensor_tensor(out=st["Tp"][:], in0=gmT[:], in1=msl, op=Alu.mult)
            st["TpT"] = mp.tile([T, T], f32, tag="TpT", name="TpT0")
            nc.vector.tensor_tensor(out=st["TpT"][:], in0=gm_f[:], in1=msu, op=Alu.mult)
            st["G"] = mp.tile([T, T], f32, tag="G", name="G0")
            nc.vector.tensor_tensor(out=st["G"][:], in0=ident, in1=st["Tp"][:], op=Alu.add)
            L_low = constp.tile([T, T], f32, tag="Llow")
            nc.vector.tensor_tensor(out=L_low[:], in0=gm_f[:], in1=mle, op=Alu.mult)
            MT = constp.tile([T, T], f32, tag="MT")

            def chain_step():
                # one squaring of the nilpotent-inverse chain, as 4 drip-ops
                # (PE matmuls; copies/adds on Act so PE never parks on them)
                def mms():
                    st["sq"] = psM.tile([T, T], f32, tag="mm", name="sq")
                    nc.tensor.matmul(st["sq"][:], st["Tp"][:], st["TpT"][:])
                    st["sq2"] = psM.tile([T, T], f32, tag="mm", name="sq2")
                    nc.tensor.matmul(st["sq2"][:], st["TpT"][:], st["Tp"][:])

                def copies():
                    st["Tp"] = mp.tile([T, T], f32, tag="Tp", name="Tpn")
                    nc.scalar.activation(st["Tp"][:], st["sq2"][:], Copy)
                    st["TpT"] = mp.tile([T, T], f32, tag="TpT", name="TpTn")
                    nc.scalar.activation(st["TpT"][:], st["sq"][:], Copy)

                def mmg():
                    st["gu"] = psM.tile([T, T], f32, tag="mm", name="gu")
                    nc.tensor.matmul(st["gu"][:], st["TpT"][:], st["G"][:])

                def addg():
                    G_n = mp.tile([T, T], f32, tag="G", name="Gn")
                    nc.vector.tensor_tensor(
                        out=G_n[:], in0=st["G"][:], in1=st["gu"][:], op=Alu.add
                    )
                    st["G"] = G_n

                return [mms, copies, mmg, addg]

            chain_ops = []
            for _k in range(6):
                chain_ops.extend(chain_step())
            chain_ops.append(
                lambda: (
                    st.__setitem__("mt", psM.tile([T, T], f32, tag="mm", name="mt")),
                    nc.tensor.matmul(st["mt"][:], L_low[:], st["G"][:]),
                )
            )
            chain_ops.append(lambda: nc.scalar.activation(MT[:], st["mt"][:], Copy))

            loop_cm = (
                tc.For_i(0, loop_stream, 1)
                if loop_stream > 1
                else contextlib.nullcontext()
            )
            def temp_sub(j, s):
                # the (tok, tag) lhsT slice for subtile s of chunk j, from
                # the engine-private tempT tile of engine j % NENG
                return t_eng[j % NENG][:, ts((j // NENG) * SUBS + s, 128)]

            with loop_cm:
                # ---- span-interleaved: temps+relus for span g, then the
                #      P matmuls for span g-1 (a full span of slack decouples
                #      the PE from the relu drain) ----
                t_eng = [
                    workp.tile(
                        [128, ((NCHUNK - e + NENG - 1) // NENG) * CHUNK],
                        bf16,
                        tag=f"teng{e}",
                        name=f"t_eng{e}",
                    )
                    for e in range(NENG)
                ]
                P_ps = psA.tile([128, XW], f32, tag="pacc")

                def emit_span_P(g):
                    xt = xt_tiles[g]
                    for cs in range(DSPAN_X * SUBS):
                        i = g * DSPAN_X * SUBS + cs
                        j, s = divmod(i, SUBS)
                        nc.tensor.matmul(
                            P_ps[:],
                            temp_sub(j, s),
                            xt[:, ts(cs, XW)],
                            start=(i == 0),
                            stop=(i == NSUB - 1),
                        )

                LEAD = 2
                for k in range(NSPAN + LEAD):
                  if k < NSPAN:
                    g = k
                    for c in range(DSPAN):
                        j = g * DSPAN + c
                        tt_in = tt_tiles[g]
                        ttp = psB.tile([128, CHUNK], f32, tag="tt")
                        for s in range(SUBS):
                            nc.tensor.matmul(
                                ttp[:, ts(s, 128)],
                                tt_in[:, :, ts(c * SUBS + s, 128)],
                                cmat[:],
                                perf_mode=DR,
                            )
                        dst = t_eng[j % NENG][:, ts(j // NENG, CHUNK)]
                        if j % NENG == 0:
                            nc.vector.tensor_scalar_max(dst, ttp[:], 0.0)
                        else:
                            nc.scalar.activation(dst, ttp[:], Relu)
                        if (
                            j >= 2
                            and j % 2 == 0
                            and (j - 2) // 2 < len(chain_ops)
                            and "nochain" not in _VARIANT
                        ):
                            chain_ops[(j - 2) // 2]()
                  if k >= LEAD:
                    emit_span_P(k - LEAD)
                if "nochain" in _VARIANT:
                    for f in chain_ops:
                        f()

            # ---- out = M @ (diag(1/r) P)  (lhsT = MT) ----
            inv_r = workp.tile([128, 1], f32, tag="invr")
            nc.vector.reciprocal(inv_r[:], P_ps[:, DS : DS + 1])
            P_sb = workp.tile([128, DS], f32, tag="Psb")
            nc.vector.tensor_scalar_mul(P_sb[:], P_ps[:, 0:DS], inv_r[:])

            o_ps = psM.tile([T, T], f32, tag="mm")
            nc.tensor.matmul(o_ps[:], MT[:], P_sb[:])
            out_sb = workp.tile([128, DS], f32, tag="outsb")
            nc.vector.tensor_copy(out_sb[:], o_ps[:])
            nc.sync.dma_start(out_d[:], out_sb[:])

    nc.compile()
    return nc


def _get_program(with_cc=True, loop_stream=1):
    key = (with_cc, loop_stream)
    if key not in _PROGRAM:
        _PROGRAM[key] = _build_program(with_cc, loop_stream)
    return _PROGRAM[key]


def _make_in_maps(inputs):
    import ml_dtypes

    bf16 = ml_dtypes.bfloat16
    x = np.asarray(inputs["inputs"], dtype=np.float32).reshape(N, D)
    t2t = np.asarray(inputs["tag_to_token"], dtype=np.float32).reshape(T, N)
    gm = np.asarray(inputs["gat_mask"], dtype=np.int32).reshape(T, T)
    # DoubleRow layout: [p, i, tok] = binary[i*64 + p, tok]
    t2t_bin = np.ascontiguousarray(
        (t2t > 0).reshape(2, 64, N).transpose(1, 0, 2)
    ).astype(ml_dtypes.float8_e4m3)
    consts = _host_consts()
    in_maps = []
    for c in range(NCORES):
        xc = x[:, c * DS : (c + 1) * DS].astype(bf16)
        xa = np.concatenate([xc, np.ones((N, 1), dtype=bf16)], axis=1)
        # (N, XW) -> span-major layout:
        # [g*128+p, (c*SUBS+s)*XW+q] = xa[((g*DSPAN+c)*SUBS+s)*128+p, q]
        xa = np.ascontiguousarray(
            xa.reshape(NSPAN_X, DSPAN_X * SUBS, 128, XW)
            .transpose(0, 2, 1, 3)
            .reshape(NSPAN_X * 128, DSPAN_X * SUBS * XW)
        )
        m = {"xs": xa, "t2t": t2t_bin, "gm": gm}
        m.update(consts)
        in_maps.append(m)
    return in_maps


def _run(inputs, trace=False, **kw):
    from concourse.bass_utils import run_bass_kernel_spmd

    nc = _get_program()
    in_maps = _make_in_maps(inputs)
    res = run_bass_kernel_spmd(
        nc, in_maps, list(range(NCORES)), trace=trace, **kw
    )
    out = np.empty((T, D), dtype=np.float32)
    for c in range(NCORES):
        out[:, c * DS : (c + 1) * DS] = np.asarray(res.results[c]["out"])
    return out.reshape(B, T, D), res


def kernel(**inputs) -> np.ndarray:
    out, _ = _run(inputs, trace=False)
    return out
